# revision 2
# baseline (speedup 1.0000x reference)
"""BitLinear (BitNet b1.58) Trainium2 kernel v2, 8-core data-parallel.

Reference computation (fp32):
    scale  = 127 / clip(max|x| over d_in, 1e-5)          (per token)
    xq     = clip(round(x*scale), -128, 127) / scale     (per-token int8 quant-dequant)
    s      = clip(mean|W|, 1e-5)
    wq     = clip(round(W/s), -1, 1) * s                 (ternary quant)
    out    = xq @ wq.T

Strategy (per core, tokens sharded 4096/core, weight replicated):
  * q = round(x*scale) via the fp32 magic-number trick; xs = q*(absmax/127)
    cast bf16 in one fused tensor_scalar.  The global weight scale s is
    folded into the ternary weights (tT = clip(round(w/s),-1,1)*s, bf16),
    so the whole x pipeline is s-independent and streams from t=0.
  * ALL transposes go through the DMA XBAR (dma_start_transpose): grouped
    4-tile xs transposes -> xsT[p, n, k, t], per-ob weight transposes ->
    tTall[p, k, o].  The PE runs ONLY matmuls.  All XBAR transposes must
    share ONE HWDGE ring: issuing transposes on both rings concurrently
    corrupts data on HW (sim-clean).
  * out^T formulation: stationary = weight block tT(k, ob), moving = xsT
    token stream in chunks [4,4,8,8,8] tiles sweeping ob -> k into
    PSUM-resident accumulators; redundant LDWEIGHTS are stripped by a
    post-pass so each (ob,k) stationary loads once per chunk.  psum is
    drained (fp32->bf16) alternating ACT/DVE and DMAd to a transposed
    bf16 output [D_OUT, TOK]; the host transposes/upcasts to fp32.
"""

import numpy as np

import concourse.bass as bass
import concourse.mybir as mybir
from concourse import tile
from concourse.bass_utils import run_bass_kernel_spmd

F32 = mybir.dt.float32
BF16 = mybir.dt.bfloat16

N_CORES = 8
B, S, D_IN, D_OUT = 4, 8192, 1024, 1024
TOKENS = B * S                     # 32768
TOK_PER_CORE = TOKENS // N_CORES   # 4096
TILES = TOK_PER_CORE // 128        # 32
KT = D_IN // 128                   # 8 contraction k-tiles
OB = D_OUT // 128                  # 8 output row blocks of W

CHUNKS = [(0, 4), (4, 4), (8, 8), (16, 8), (24, 8)]   # (tile0, ntiles) chunks

EPS = 1e-5
QMAX = 127.0
MAGIC = 12582912.0                     # 1.5 * 2**23 -> RNE integer rounding


def _split_multiwaits(nc):
    """walrus here encodes at most ONE sem wait per instruction; Tile's tail
    drain (and occasionally other insts) carry several.  Split extras into
    single-wait NOPs on the same engine, preserving order."""
    for f in nc.m.functions:
        for bb in f.blocks:
            insts = list(bb.instructions)
            if not any(
                i.sync_info and len(i.sync_info.on_wait) > 1 for i in insts
            ):
                continue
            new = []
            for ins in insts:
                si = ins.sync_info
                if si and len(si.on_wait) > 1:
                    waits = list(si.on_wait)
                    for j, w in enumerate(waits[:-1]):
                        nop = mybir.InstNoOp(
                            name=f"{ins.name}_wsp{j}", ins=[], outs=[]
                        )
                        nop.engine = ins.engine
                        nop.sync_info = mybir.SyncInfo(on_wait=[w], on_update=[])
                        new.append(nop)
                    ins.sync_info = mybir.SyncInfo(
                        on_wait=[waits[-1]], on_update=list(si.on_update)
                    )
                new.append(ins)
            bb.instructions = new


def _dedup_ldweights(nc):
    """Drop InstLdweights whose stationary AP is identical to the PE array's
    current contents (consecutive same-stationary matmul groups).  Non-self-
    loading matmuls then reuse the loaded weights.  Waits/updates on dropped
    loads are preserved as single-wait NOPs."""
    harmless = (
        mybir.InstNoOp, mybir.InstEventSemaphore, mybir.InstDrain,
        mybir.InstRegisterMove,
    )

    def sig_of(ap):
        return (ap.memref, ap.offset, str(ap.ap), str(ap.dtype))

    ndrop = 0
    for f in nc.m.functions:
        for bb in f.blocks:
            last_sig = None
            new = []
            for ins in bb.instructions:
                if ins.engine != mybir.EngineType.PE:
                    new.append(ins)
                    continue
                if isinstance(ins, mybir.InstLdweights):
                    sig = sig_of(ins.ins[0])
                    if sig == last_sig:
                        ndrop += 1
                        si = ins.sync_info
                        waits = list(si.on_wait) if si else []
                        ups = list(si.on_update) if si else []
                        for j, w in enumerate(waits):
                            nop = mybir.InstNoOp(
                                name=f"{ins.name}_lw{j}", ins=[], outs=[]
                            )
                            nop.engine = ins.engine
                            nop.sync_info = mybir.SyncInfo(
                                on_wait=[w],
                                on_update=ups if j == len(waits) - 1 else [],
                            )
                            new.append(nop)
                        if not waits and ups:
                            nop = mybir.InstNoOp(
                                name=f"{ins.name}_lwu", ins=[], outs=[]
                            )
                            nop.engine = ins.engine
                            nop.sync_info = mybir.SyncInfo(on_wait=[], on_update=ups)
                            new.append(nop)
                        continue
                    last_sig = sig
                elif isinstance(ins, mybir.InstMatmult):
                    if ins.ins[1].dtype in (mybir.dt.float32, mybir.dt.float32r):
                        last_sig = None  # self-loading matmul clobbers the array
                elif not isinstance(ins, harmless):
                    last_sig = None
                new.append(ins)
            bb.instructions = new
    return ndrop


def build_program(split=True):
    nc = bass.Bass(trn_type="TRN2")
    x_d = nc.dram_tensor("x", [TOK_PER_CORE, D_IN], F32, kind="ExternalInput")
    w_d = nc.dram_tensor("weight", [D_OUT, D_IN], F32, kind="ExternalInput")
    o_d = nc.dram_tensor("out", [D_OUT, TOK_PER_CORE], BF16, kind="ExternalOutput")

    Copy = mybir.ActivationFunctionType.Copy
    Abs = mybir.ActivationFunctionType.Abs
    AX = mybir.AxisListType.X
    op = mybir.AluOpType

    with tile.TileContext(nc) as tc:
        from contextlib import ExitStack

        with ExitStack() as ctx:
            singles = ctx.enter_context(tc.tile_pool(name="singles", bufs=1))

            ones_col = singles.tile([128, 1], F32)
            nc.vector.memset(ones_col[:], 1.0)
            ones_row = singles.tile([1, 128], F32)
            nc.vector.memset(ones_row[:], 1.0)
            bc2 = singles.tile([128, 2], F32)    # [s, 1/s] broadcast to 128 parts

            # block-transposed activations: xsT[p, n, k, t] = xs_n[t, 128k+p]
            xsT = singles.tile([128, TILES, KT, 128], BF16, name="xsT", tag="xsT")
            # transposed ternary weights: tTall[p, k, o] = wq[o, 128k+p]
            tTall = singles.tile([128, KT, D_OUT], BF16, name="tTall", tag="tTall")

            xpool = ctx.enter_context(tc.tile_pool(name="xpool", bufs=2))
            xmpool = ctx.enter_context(tc.tile_pool(name="xmpool", bufs=4))
            xspool = ctx.enter_context(tc.tile_pool(name="xspool", bufs=2))
            outpool = ctx.enter_context(tc.tile_pool(name="outpool", bufs=8))
            smpool = ctx.enter_context(tc.tile_pool(name="smpool", bufs=10))
            GRP = 4  # tiles per x DMA / transpose group

            live = {}

            def a_dma(g):
                # grouped 4-tile x DMA (2 MB: amortizes per-DMA latency)
                xg = xpool.tile([128, GRP, D_IN], F32, tag="x")
                src = x_d[g * GRP * 128:(g + 1) * GRP * 128, :].rearrange(
                    "(j p) d -> p j d", p=128
                )
                nc.sync.dma_start(xg[:, :, :], src)
                live[("xg", g)] = xg
                for j in range(GRP):
                    n = g * GRP + j
                    am = smpool.tile([128, 1], F32, tag="am")
                    nc.vector.tensor_reduce(
                        am[:], xg[:, j, :], axis=AX, op=op.max,
                        apply_absolute_value=True,
                    )
                    ram = smpool.tile([128, 1], F32, tag="ram")
                    nc.vector.reciprocal(ram[:], am[:])
                    scl = smpool.tile([128, 1], F32, tag="scl")
                    nc.vector.tensor_scalar(scl[:], ram[:], QMAX, None, op0=op.mult)
                    coef = smpool.tile([128, 1], F32, tag="coef")
                    nc.vector.tensor_scalar(coef[:], am[:], 1.0 / QMAX, None, op0=op.mult)
                    live[("scl", n)] = scl
                    live[("coef", n)] = coef

            def a_quant_group(g):
                # xs = q * (absmax/127); s-free, runs from t=0.  One XBAR
                # transpose per 4-tile group.
                xg = live.pop(("xg", g))
                xsg = xspool.tile([128, GRP, D_IN], BF16, tag="xs")
                for j in range(GRP):
                    n = g * GRP + j
                    scl = live.pop(("scl", n))
                    coef = live.pop(("coef", n))
                    xm = xmpool.tile([128, D_IN], F32, tag="xm")
                    nc.scalar.activation(
                        xm[:], xg[:, j, :], Copy, bias=MAGIC, scale=scl[:]
                    )
                    nc.vector.tensor_scalar(
                        xsg[:, j, :], xm[:], -MAGIC, coef[:],
                        op0=op.add, op1=op.mult,
                    )
                nc.sync.dma_start_transpose(
                    xsT[:, g * GRP:(g + 1) * GRP, :, :], xsg[:, :, :]
                )

            # ---------------- weight phase (interleaved with x ramp) -------
            with (
                tc.tile_pool(name="wpool", bufs=1) as wpool,
                tc.tile_pool(name="wtmp", bufs=2) as wtmp,
                tc.tile_pool(name="wps", bufs=2, space="PSUM") as wps,
            ):
                # weight in TWO grouped DMAs on the ACT HWDGE ring (each
                # spreads across all 16 SDMA slots; x stays on the SP ring)
                w_all = wpool.tile([128, OB, D_IN], F32, name="w_all", tag="w_all")
                w_src = w_d[:, :].rearrange("(a p) d -> p a d", p=128)
                half = OB // 2
                nc.scalar.dma_start(w_all[:, :half, :], w_src[:, :half, :])
                nc.scalar.dma_start(w_all[:, half:, :], w_src[:, half:, :])
                w_t = [w_all[:, ob, :] for ob in range(OB)]
                a_dma(0)
                # first quants BEFORE the w-gated colsum ops so xm/xs don't
                # queue behind them on ACT/DVE (x path is s-independent now)
                a_quant_group(0)

                # |w| column sums: split ACT (Abs+accum) / DVE (abs-reduce)
                from contextlib import ExitStack as _ES
                wctx = _ES()
                wctx.enter_context(tc.high_priority(offset=60))
                colsum = wpool.tile([128, OB], F32)
                for ob in range(OB):
                    if ob % 2 == 0:
                        wabs = xmpool.tile([128, D_IN], F32, name="wabs", tag="xm")
                        nc.scalar.activation(
                            wabs[:], w_t[ob], Abs, accum_out=colsum[:, ob:ob + 1]
                        )
                    else:
                        nc.vector.tensor_reduce(
                            colsum[:, ob:ob + 1], w_t[ob], axis=AX, op=op.add,
                            apply_absolute_value=True,
                        )
                colsum2 = wpool.tile([128, 1], F32)
                nc.vector.tensor_reduce(colsum2[:], colsum[:], axis=AX, op=op.add)

                ps_m1 = wps.tile([1, 2], F32, name="ps_m1", tag="ps_m")
                nc.tensor.matmul(ps_m1[0:1, 0:1], ones_col[:], colsum2[:])
                pair = wpool.tile([1, 2], F32)
                nc.scalar.activation(pair[:, 0:1], ps_m1[0:1, 0:1], Copy, scale=1.0 / (D_OUT * D_IN))
                nc.vector.tensor_scalar_max(pair[:, 0:1], pair[:, 0:1], EPS)
                nc.vector.reciprocal(pair[:, 1:2], pair[:, 0:1])
                ps_m2 = wps.tile([128, 2], F32, name="ps_m2", tag="ps_m")
                nc.tensor.matmul(ps_m2[:], ones_row[:], pair[:])
                nc.scalar.copy(bc2[:], ps_m2[:])

                # ternarize in natural layout (bf16 {-1,0,1}), then XBAR
                # transpose each ob tile into the tTall[d, k, o] slab.
                # Interleave 1:1 with early token quants so neither stream
                # monopolizes the DVE queue.
                for ob in range(OB):
                    y0 = xmpool.tile([128, D_IN], F32, name="y0", tag="xm")
                    nc.scalar.activation(
                        y0[:], w_t[ob], Copy, bias=MAGIC, scale=bc2[:, 1:2]
                    )
                    t1 = xmpool.tile([128, D_IN], F32, name="t1", tag="xm")
                    nc.vector.tensor_scalar(
                        t1[:], y0[:], -MAGIC, 1.0, op0=op.add, op1=op.min
                    )
                    wq = wtmp.tile([128, D_IN], BF16, name="wq", tag="wq")
                    nc.vector.tensor_scalar(
                        wq[:], t1[:], -1.0, bc2[:, 0:1], op0=op.max, op1=op.mult
                    )
                    nc.sync.dma_start_transpose(
                        tTall[:, :, ob * 128:(ob + 1) * 128], wq[:]
                    )
                wctx.close()
                a_dma(1)

            pso = ctx.enter_context(tc.tile_pool(name="pso", bufs=4, space="PSUM"))

            # feeder: a_dma/a_quant emitted at chunk START only, one full
            # chunk ahead, so feeder work never queues behind psum drains
            # (which wait on PE progress) on ACT/DVE.
            NGRP = TILES // GRP
            next_dma = [2]
            next_quant = [1]

            def feed_one():
                if next_dma[0] < NGRP and next_dma[0] - next_quant[0] < 2:
                    a_dma(next_dma[0])
                    next_dma[0] += 1
                if next_quant[0] < NGRP and next_quant[0] < next_dma[0]:
                    a_quant_group(next_quant[0])
                    next_quant[0] += 1

            drain_flip = [0]
            for ci, (t0, nt) in enumerate(CHUNKS):
                nxt_end = (CHUNKS[ci + 1][0] + CHUNKS[ci + 1][1]
                           if ci + 1 < len(CHUNKS) else TILES)
                while next_quant[0] < nxt_end // GRP:
                    feed_one()
                T = nt * 128
                segs = nt // 4
                halves = [(t0 + h * 8, min(nt - h * 8, 8))
                          for h in range((nt + 7) // 8)]
                for ob in range(OB):
                    pss = [pso.tile([128, 1024], F32, tag="ps", name=f"ps{hi}") for hi in range(len(halves))]
                    for k in range(KT):
                        for psf, (h0, hn) in zip(pss, halves):
                            for sg in range(hn // 4):
                                nc.tensor.matmul(
                                    psf[:, sg * 512:(sg + 1) * 512],
                                    tTall[:, k, ob * 128:(ob + 1) * 128],
                                    xsT[:, h0 + sg * 4:h0 + (sg + 1) * 4, k, :],
                                    start=(k == 0),
                                    stop=(k == KT - 1),
                                )
                    for psf, (h0, hn) in zip(pss, halves):
                        hT = hn * 128
                        osb = outpool.tile([128, 1024], BF16, tag="osb")
                        if drain_flip[0] % 2 == 0:
                            nc.scalar.activation(osb[:, :hT], psf[:, :hT], Copy)
                        else:
                            nc.vector.tensor_copy(osb[:, :hT], psf[:, :hT])
                        drain_flip[0] += 1
                        nc.sync.dma_start(
                            o_d[ob * 128:(ob + 1) * 128, h0 * 128:h0 * 128 + hT],
                            osb[:, :hT],
                        )

    _dedup_ldweights(nc)
    if split:
        _split_multiwaits(nc)
    return nc


_NC_CACHE = None


def _get_nc():
    global _NC_CACHE
    if _NC_CACHE is None:
        _NC_CACHE = build_program()
    return _NC_CACHE


def kernel(x: np.ndarray, weight: np.ndarray, trace: bool = False):
    assert x.shape == (B, S, D_IN) and weight.shape == (D_OUT, D_IN)
    nc = _get_nc()
    xf = np.ascontiguousarray(x.reshape(TOKENS, D_IN), dtype=np.float32)
    w = np.ascontiguousarray(weight, dtype=np.float32)
    in_maps = [
        {
            "x": xf[c * TOK_PER_CORE:(c + 1) * TOK_PER_CORE],
            "weight": w,
        }
        for c in range(N_CORES)
    ]
    res = run_bass_kernel_spmd(nc, in_maps, core_ids=list(range(N_CORES)), trace=trace)
    kernel.last_results = res
    outs = [
        np.asarray(res.results[c]["out"]).astype(np.float32)  # [D_OUT, TOK_PER_CORE]
        for c in range(N_CORES)
    ]
    out = np.concatenate(outs, axis=1).T  # [TOKENS, D_OUT]
    return np.ascontiguousarray(out).reshape(B, S, D_OUT)


kernel.last_results = None
